# revision 23
# baseline (speedup 1.0000x reference)
"""Trainium2 Bass kernel for nn_AttentionPool (segment softmax-pool over gene/spot edges).

Math: out[g] = (sum_{s in S_g} e_s * emb[s]) / (sum_{s in S_g} e_s),
      e_s = exp(logit_s - c),  logit = tanh(emb @ W.T + b) @ v
where S_g is the *set* of distinct spots expressing gene g (duplicate edges
count once), and empty genes produce 0. Any shift c cancels in the ratio;
c = 5.0 (> max logit ~3.96 for this problem's xavier init) keeps every e_s
in fp8e4m3 range with wide margin on both ends.

Sharding: 2500 genes per core x 8 cores (padded to 2560 = 5 groups of 512).
The masked pool runs with X = e*emb as the *stationary* fp8 operand in
DoubleRowSwInterleave layout (K=256 spots per pass) and the dense fp8 {0,1}
occupancy mask streaming as the moving operand, 512 genes per matmul - so
the big streamed-once mask never pays LDWEIGHTS. Per 256-spot chunk and gene
group three passes accumulate: Xhi (128 dims), Xlo (64x residual, bf16-level
accuracy), and the e hi/lo pair (denominator, 2 PSUM rows). num lands
transposed [d, gene]; rinv rows are broadcast across partitions with a
rank-1 ones matmul and the host untransposes the output slabs.
"""

import sys

sys.path.insert(0, "/opt/trn_rl_repo")

import numpy as np
import ml_dtypes

import concourse.mybir as mybir
import concourse.tile as tile
from concourse import bacc
from concourse.bass import ts
from concourse.bass_utils import run_bass_kernel_spmd
from concourse.bass_interp import get_hw_module

F32 = mybir.dt.float32
F32R = mybir.dt.float32r
BF16 = mybir.dt.bfloat16
F8 = mybir.dt.float8e4

N_SPOTS = 4096
N_GENES = 20000
D = 128
N_CORES = 8
G_PER = N_GENES // N_CORES  # 2500
P = 128
KCH = N_SPOTS // P  # 32 spot chunks of 128
KK = KCH // 2  # 16 double-chunks of 256 for DoubleRow
LO_SCALE = 64.0  # keeps fp8 lo residuals out of the subnormal range
C_SHIFT = 5.0  # logit shift; exact value cancels in the num/den ratio
NG = 8  # X built in 8 groups of 4 chunks (= 2 double-chunks)
GS = KCH // NG  # 4
GG = 512  # genes per group (psum bank width)
NQ = 4  # mask slab tiles per gene group (4 double-chunks each)


def build_nc(T):
    """Build the single-core Bass program (SPMD across 8 cores).

    T = gene groups of 512 per core (5 for the real problem: 2560 padded).
    """
    nc = bacc.Bacc("TRN2", target_bir_lowering=False, debug=False, num_devices=N_CORES)

    maskb = nc.dram_tensor("maskb", [T, NQ, P, (KK // NQ) * 2 * GG], F8, kind="ExternalInput")
    embT = nc.dram_tensor("embT", [P, N_SPOTS], BF16, kind="ExternalInput")
    embc = nc.dram_tensor("embc", [P, KCH * D], BF16, kind="ExternalInput")
    wt = nc.dram_tensor("wt", [D, D], BF16, kind="ExternalInput")
    bb = nc.dram_tensor("bb", [D, 1], F32, kind="ExternalInput")
    vv = nc.dram_tensor("vv", [D, 1], BF16, kind="ExternalInput")
    w2 = nc.dram_tensor("w2", [2, 1], F32, kind="ExternalInput")
    out2 = nc.dram_tensor("out2", [T, P, GG], F32, kind="ExternalOutput")

    with tile.TileContext(nc) as tc:
        with (
            tc.tile_pool(name="const", bufs=1) as constp,
            tc.tile_pool(name="xfp", bufs=1) as xfp,
            tc.tile_pool(name="maskp", bufs=T * NQ) as maskp,
            tc.tile_pool(name="outp", bufs=3) as outp,
            tc.tile_pool(name="php", bufs=2, space="PSUM") as php,
            tc.tile_pool(name="pep", bufs=1, space="PSUM") as pep,
            tc.tile_pool(name="pap", bufs=2, space="PSUM") as pap,
            tc.tile_pool(name="pbp", bufs=2, space="PSUM") as pbp,
            tc.tile_pool(name="pcp", bufs=1, space="PSUM") as pcp,
        ):
            # ---- constants into SBUF ----
            wt_sb = constp.tile([P, D], BF16)
            nc.sync.dma_start(out=wt_sb[:], in_=wt[:])
            b_sb = constp.tile([P, 1], F32)
            nc.sync.dma_start(out=b_sb[:], in_=bb[:])
            v_sb = constp.tile([P, 1], BF16)
            nc.sync.dma_start(out=v_sb[:], in_=vv[:])
            HS = N_SPOTS // 2
            embT_a = constp.tile([P, HS], BF16)
            embT_b = constp.tile([P, HS], BF16)
            nc.gpsimd.dma_start(out=embT_a[:], in_=embT[:, 0:HS])
            nc.sync.dma_start(out=embT_b[:], in_=embT[:, HS:])
            HC = KCH * D // 2
            embc_a = constp.tile([P, HC], BF16)
            embc_b = constp.tile([P, HC], BF16)
            nc.gpsimd.dma_start(out=embc_a[:], in_=embc[:, 0:HC])
            nc.scalar.dma_start(out=embc_b[:], in_=embc[:, HC:])

            def embT_cols(lo, width):
                if lo < HS:
                    return embT_a[:, lo : lo + width]
                return embT_b[:, lo - HS : lo - HS + width]

            negc_sb = constp.tile([P, 1], F32)
            nc.vector.memset(negc_sb[:], -C_SHIFT)
            w2_sb = constp.tile([2, 1], F32)
            nc.sync.dma_start(out=w2_sb[:], in_=w2[:])
            ones_sb = constp.tile([1, P], F32)
            nc.vector.memset(ones_sb[:], 1.0)

            th_sb = constp.tile([P, N_SPOTS], BF16)  # tanh(W h + b).T  [j, s]
            e_sb = constp.tile([P, KCH], F32)  # e in spot-partition layout
            # per-group stationary X tiles in SwInterleave layout:
            # flat free (kkl, 2j+i) holds ktile i, reversed column j
            xwa = [constp.tile([P, 2 * 2 * D], F8, name=f"xwa{g}") for g in range(NG)]
            xwb = [constp.tile([P, 2 * 2 * D], F8, name=f"xwb{g}") for g in range(NG)]
            xwc = [constp.tile([P, 2 * P * 2], F8, name=f"xwc{g}") for g in range(NG)]

            for g in range(NG):
                nc.vector.memset(xwc[g][:], 0.0)

            # ---- mask slab DMAs (gpsimd software ring, after emb loads) ----
            mts = []
            for gg in range(T):
                for q in range(NQ):
                    mt = maskp.tile(
                        [P, (KK // NQ) * 2 * GG], F8, name=f"mt{gg}_{q}", tag="mt"
                    )
                    nc.gpsimd.dma_start(out=mt[:], in_=maskb[gg, q])
                    mts.append(mt)

            # ---- prologue: th = tanh(W@emb.T + b), logits, e ----
            NCH = N_SPOTS // 512  # 8 th chunks of 512 spots
            pe = pep.tile([P, KCH], F32)
            for c in range(NCH):
                ph = php.tile([P, 512], F32, tag="ph")
                nc.tensor.matmul(
                    out=ph[:], lhsT=wt_sb[:], rhs=embT_cols(c * 512, 512),
                    start=True, stop=True,
                )
                nc.scalar.activation(
                    out=th_sb[:, ts(c, 512)], in_=ph[:],
                    func=mybir.ActivationFunctionType.Tanh, bias=b_sb[:, 0:1],
                )
                # logits in spot-partition layout: [128 s, 1] = th_k.T @ v
                for k in range(4 * c, 4 * c + 4):
                    nc.tensor.matmul(
                        out=pe[:, k : k + 1], lhsT=th_sb[:, ts(k, P)], rhs=v_sb[:],
                        start=True, stop=True,
                    )
                nc.scalar.activation(
                    out=e_sb[:, 4 * c : 4 * c + 4], in_=pe[:, 4 * c : 4 * c + 4],
                    func=mybir.ActivationFunctionType.Exp, bias=negc_sb[:, 0:1],
                )

            # ---- X as fp8 hi + 64x lo in SwInterleave weight layout ----
            # embc comes d-reversed from the host, cancelling SwInterleave's
            # column reversal, so psum rows come out in natural d order.
            xf = xfp.tile([P, KCH * D], F32)
            xd = xfp.tile([P, KCH * D], F32)
            xf3 = xf[:].rearrange("p (k j) -> p k j", j=D)
            xd3 = xd[:].rearrange("p (k j) -> p k j", j=D)
            # chunk-major <-> interleaved views of the same buffers
            xfv = xf[:].rearrange("p (g kkl i j) -> p g kkl j i", g=NG, kkl=2, i=2)
            xdv = xd[:].rearrange("p (g kkl i j) -> p g kkl j i", g=NG, kkl=2, i=2)
            xf4 = xf[:].rearrange("p (g kkl i j) -> p g kkl i j", g=NG, kkl=2, i=2)
            xd4 = xd[:].rearrange("p (g kkl i j) -> p g kkl i j", g=NG, kkl=2, i=2)
            emb3a = embc_a[:].rearrange("p (k d) -> p k d", d=D)
            emb3b = embc_b[:].rearrange("p (k d) -> p k d", d=D)
            e3 = e_sb[:].rearrange("p k -> p k ()")
            e4 = e_sb[:].rearrange("p (g kkl i) -> p g kkl i", kkl=2, i=2)
            for g in range(NG):
                ks = slice(g * GS, (g + 1) * GS)
                if g < NG // 2:
                    embsrc = emb3a[:, ks, :]
                else:
                    embsrc = emb3b[:, slice(g * GS - KCH // 2, (g + 1) * GS - KCH // 2), :]
                ebc = e3[:, ks, :].to_broadcast([P, GS, D])
                wav = xwa[g][:].rearrange("p (kkl j i) -> p kkl j i", j=D, i=2)
                wbv = xwb[g][:].rearrange("p (kkl j i) -> p kkl j i", j=D, i=2)
                wa_ch = xwa[g][:].rearrange("p (kkl j i) -> p kkl i j", j=D, i=2)
                nc.vector.tensor_mul(out=xf3[:, ks, :], in0=embsrc, in1=ebc)
                nc.scalar.activation(
                    out=wav[:], in_=xfv[:, g],
                    func=mybir.ActivationFunctionType.Copy,
                )
                nc.vector.tensor_sub(out=xd4[:, g], in0=xf4[:, g], in1=wa_ch)
                nc.scalar.activation(
                    out=wbv[:], in_=xdv[:, g],
                    func=mybir.ActivationFunctionType.Copy, scale=LO_SCALE,
                )
                # e hi/lo pair tile: per kkl flat [j(2), i(2)]; row order is
                # j-reversed so j=1 holds e_hi (den row 0), j=0 holds e_lo
                wcv = xwc[g][:].rearrange("p (kkl j i) -> p kkl j i", j=P, i=2)
                nc.vector.tensor_copy(out=wcv[:, :, P - 1, :], in_=e4[:, g])
                ed = outp.tile([P, GS], F32, tag="ed")
                ed3 = ed[:].rearrange("p (kkl i) -> p kkl i", i=2)
                nc.vector.tensor_sub(out=ed3[:], in0=e4[:, g], in1=wcv[:, :, P - 1, :])
                nc.scalar.activation(
                    out=wcv[:, :, P - 2, :], in_=ed3[:],
                    func=mybir.ActivationFunctionType.Copy, scale=LO_SCALE,
                )

            # ---- main loop: per 512-gene group, mask streams as rhs ----
            KQ = KK // NQ  # 4 double-chunks per mask tile
            for gg in range(T):
                psa = pap.tile([P, GG], F32, name=f"psa{gg}", tag="psa")
                psb = pbp.tile([P, GG], F32, name=f"psb{gg}", tag="psb")
                psc = pcp.tile([P, GG], F32, name=f"psc{gg}", tag="psc")
                for kk in range(KK):
                    g, kkl = kk // 2, kk % 2
                    q, kq = kk // KQ, kk % KQ
                    mt4 = mts[gg * NQ + q][:].rearrange(
                        "p (kq i n) -> p kq i n", i=2, n=GG
                    )
                    rhs = mt4[:, kq]
                    wav = xwa[g][:].rearrange("p (kkl j i) -> p kkl j i", j=D, i=2)
                    wbv = xwb[g][:].rearrange("p (kkl j i) -> p kkl j i", j=D, i=2)
                    wcv = xwc[g][:].rearrange("p (kkl j i) -> p kkl j i", j=P, i=2)
                    st = dict(start=(kk == 0), stop=(kk == KK - 1))
                    pm = mybir.MatmulPerfMode.DoubleRowSwInterleave
                    nc.tensor.matmul(
                        out=psa[:], lhsT=wav[:, kkl], rhs=rhs, perf_mode=pm, **st
                    )
                    nc.tensor.matmul(
                        out=psb[:], lhsT=wbv[:, kkl], rhs=rhs, perf_mode=pm, **st
                    )
                    nc.tensor.matmul(
                        out=psc[:], lhsT=wcv[:, kkl], rhs=rhs, perf_mode=pm, **st
                    )
                # epilogue: num = psa + psb/64 (transposed [d, g]); den rows
                s1 = outp.tile([P, GG], F32, tag="s1")
                nc.scalar.activation(
                    out=s1[:], in_=psb[:],
                    func=mybir.ActivationFunctionType.Copy, scale=1.0 / LO_SCALE,
                )
                s2 = outp.tile([P, GG], F32, tag="s2")
                nc.vector.tensor_add(out=s2[:], in0=s1[:], in1=psa[:])
                dsb = outp.tile([2, GG], F32, tag="dsb")
                nc.scalar.activation(
                    out=dsb[:], in_=psc[0:2, :],
                    func=mybir.ActivationFunctionType.Copy,
                )
                # den = dsb[0] + dsb[1]/64 via a K=2 matmul (PSUM reads must
                # start at partition 0, so rows can't be sliced directly)
                pden = pcp.tile([1, GG], F32, name=f"pden{gg}", tag="psc")
                nc.tensor.matmul(
                    out=pden[:], lhsT=w2_sb[:], rhs=dsb[:], start=True, stop=True
                )
                dmx = outp.tile([1, GG], F32, tag="dmx")
                nc.vector.tensor_scalar_max(out=dmx[:], in0=pden[0:1, :], scalar1=1e-37)
                rv = outp.tile([1, GG], F32, tag="rv")
                nc.vector.reciprocal(out=rv[:], in_=dmx[:])
                # broadcast rinv across partitions with a rank-1 ones matmul
                pr = pcp.tile([P, GG], F32, name=f"pr{gg}", tag="psc")
                nc.tensor.matmul(
                    out=pr[:], lhsT=ones_sb[:], rhs=rv[:], start=True, stop=True
                )
                o = outp.tile([P, GG], F32, tag="o")
                nc.vector.tensor_mul(out=o[:], in0=s2[:], in1=pr[:])
                nc.sync.dma_start(out=out2[gg], in_=o[:])

    nc.compile()
    return nc


def prep_inputs(spot_emb, W, b, v, gene_ids, spot_ids, T):
    """Host marshaling: shared bf16/f32 operands + per-core fp8 mask slabs."""
    emb = np.ascontiguousarray(np.asarray(spot_emb, dtype=np.float32))
    W = np.asarray(W, dtype=np.float32)
    b = np.asarray(b, dtype=np.float32)
    v = np.asarray(v, dtype=np.float32)
    gene_ids = np.asarray(gene_ids).astype(np.int64)
    spot_ids = np.asarray(spot_ids).astype(np.int64)

    bf = ml_dtypes.bfloat16
    # embc with the d axis reversed (cancels SwInterleave column reversal)
    shared = {
        "embc": np.ascontiguousarray(
            emb[:, ::-1].reshape(KCH, P, D).transpose(1, 0, 2).reshape(P, KCH * D).astype(bf)
        ),
        "embT": np.ascontiguousarray(emb.T.astype(bf)),
        "wt": np.ascontiguousarray(W.T.astype(bf)),
        "bb": np.ascontiguousarray(b.reshape(D, 1)),
        "vv": np.ascontiguousarray(v.reshape(D, 1).astype(bf)),
        "w2": np.array([[1.0], [1.0 / LO_SCALE]], dtype=np.float32),
    }

    # Dense 0/1 occupancy mask (set semantics: duplicate edges collapse),
    # per-core padded layout: core c's genes at rows [c*2560, c*2560+2500).
    g_pad = T * GG
    M = np.zeros((N_CORES * g_pad, N_SPOTS), dtype=bool)
    pad_rows = (gene_ids // G_PER) * g_pad + (gene_ids % G_PER)
    M[pad_rows, spot_ids] = True
    # [c, gg*512+g, kk*256 + i*128 + p] -> [c, gg, q, p, kq, i, g]
    KQ = KK // NQ
    Mb = (
        M.reshape(N_CORES, T, GG, NQ, KQ, 2, P)
        .transpose(0, 1, 3, 6, 4, 5, 2)
    )
    Mf8 = (np.ascontiguousarray(Mb).astype(np.uint8) * 0x38).view(
        ml_dtypes.float8_e4m3
    ).reshape(N_CORES, T, NQ, P, KQ * 2 * GG)
    return [{"maskb": Mf8[c], **shared} for c in range(N_CORES)]


_NC_CACHE = {}


def run(spot_emb, W, b, v, gene_ids, spot_ids, trace=False, **hw_kwargs):
    T = (G_PER + GG - 1) // GG  # 5
    if T not in _NC_CACHE:
        nc = build_nc(T)
        nc.m = get_hw_module(nc.m)
        _NC_CACHE[T] = nc
    nc = _NC_CACHE[T]
    in_maps = prep_inputs(spot_emb, W, b, v, gene_ids, spot_ids, T)
    res = run_bass_kernel_spmd(
        nc, in_maps, core_ids=list(range(N_CORES)), trace=trace, **hw_kwargs
    )
    outs = [
        np.ascontiguousarray(
            np.asarray(res.results[c]["out2"], dtype=np.float32)
            .reshape(T, P, GG)
            .transpose(0, 2, 1)
        ).reshape(T * GG, D)[:G_PER]
        for c in range(N_CORES)
    ]
    full = np.concatenate(outs, axis=0)
    return full, res


def kernel(spot_emb, W, b, v, gene_ids, spot_ids, n_genes):
    n_genes = int(n_genes)
    assert n_genes == N_GENES, f"kernel hardcodes n_genes={N_GENES}, got {n_genes}"
    full, _ = run(spot_emb, W, b, v, gene_ids, spot_ids, trace=False)
    return full


# revision 24
# speedup vs baseline: 1.3773x; 1.3773x over previous
"""Trainium2 Bass kernel for nn_AttentionPool (segment softmax-pool over gene/spot edges).

Math: out[g] = (sum_{s in S_g} e_s * emb[s]) / (sum_{s in S_g} e_s),
      e_s = exp(logit_s - c),  logit = tanh(emb @ W.T + b) @ v
where S_g is the *set* of distinct spots expressing gene g (duplicate edges
count once), and empty genes produce 0. Any shift c cancels in the ratio;
c = 5.0 (> max logit ~3.96 for this problem's xavier init) keeps every e_s
in fp8e4m3 range with wide margin on both ends.

Sharding: 2500 genes per core x 8 cores (padded to 2560 = 20 tiles of 128).
Host marshals the edge list into each core's dense fp8 {0,1} mask slab in the
DoubleRowSwInterleave weight layout (pair-interleaved ktiles, gene columns
reversed) so each [128,2,128] chunk is a K=256 matmul lhsT at 0.5 cycles/row.
X = [e*emb | e] is carried as fp8 hi plus 64x-scaled lo residual, merged as
258 rhs columns per chunk so one LDWEIGHTS serves both; the two PSUM column
blocks are recombined as hi + lo/64, giving bf16-level accuracy at 2x rate.
Main loop runs 4 gene tiles per PSUM generation so the tensor engine can
interleave across tiles while X groups are still being produced.
"""

import sys

sys.path.insert(0, "/opt/trn_rl_repo")

import numpy as np
import ml_dtypes

import concourse.mybir as mybir
import concourse.tile as tile
from concourse import bacc
from concourse.bass import ts
from concourse.bass_utils import run_bass_kernel_spmd
from concourse.bass_interp import get_hw_module

F32 = mybir.dt.float32
BF16 = mybir.dt.bfloat16
F8 = mybir.dt.float8e4

N_SPOTS = 4096
N_GENES = 20000
D = 128
N_CORES = 8
G_PER = N_GENES // N_CORES  # 2500
P = 128
KCH = N_SPOTS // P  # 32 spot chunks of 128
KK = KCH // 2  # 16 double-chunks of 256 for DoubleRow
NX = D + 1  # X columns: [e*emb | e]
LO_SCALE = 64.0  # keeps fp8 lo residuals out of the subnormal range
C_SHIFT = 5.0  # logit shift; exact value cancels in the num/den ratio
NG = 8  # X built in 8 groups of 4 chunks
GS = KCH // NG  # 4
TG = 4  # gene tiles per PSUM generation


def build_nc(T):
    """Build the single-core Bass program (SPMD across 8 cores).

    T = number of 128-gene tiles per core (20 for the real problem).
    """
    nc = bacc.Bacc("TRN2", target_bir_lowering=False, debug=False, num_devices=N_CORES)

    maskf = nc.dram_tensor("maskf", [T, P, KCH * P], F8, kind="ExternalInput")
    embT = nc.dram_tensor("embT", [P, N_SPOTS], BF16, kind="ExternalInput")
    embc = nc.dram_tensor("embc", [P, KCH * D], BF16, kind="ExternalInput")
    wt = nc.dram_tensor("wt", [D, D], BF16, kind="ExternalInput")
    bb = nc.dram_tensor("bb", [D, 1], F32, kind="ExternalInput")
    vv = nc.dram_tensor("vv", [D, 1], BF16, kind="ExternalInput")
    out = nc.dram_tensor("out", [T, P, D], F32, kind="ExternalOutput")

    with tile.TileContext(nc) as tc:
        with (
            tc.tile_pool(name="const", bufs=1) as constp,
            tc.tile_pool(name="xfp", bufs=1) as xfp,
            tc.tile_pool(name="maskp", bufs=T) as maskp,
            tc.tile_pool(name="outp", bufs=3) as outp,
            tc.tile_pool(name="php", bufs=2, space="PSUM") as php,
            tc.tile_pool(name="pep", bufs=1, space="PSUM") as pep,
            tc.tile_pool(name="ptp", bufs=TG, space="PSUM") as ptp,
        ):
            # ---- constants into SBUF (sync ring; scalar/ACT stays clean) ----
            wt_sb = constp.tile([P, D], BF16)
            nc.sync.dma_start(out=wt_sb[:], in_=wt[:])
            b_sb = constp.tile([P, 1], F32)
            nc.sync.dma_start(out=b_sb[:], in_=bb[:])
            v_sb = constp.tile([P, 1], BF16)
            nc.sync.dma_start(out=v_sb[:], in_=vv[:])
            HS = N_SPOTS // 2
            embT_a = constp.tile([P, HS], BF16)
            embT_b = constp.tile([P, HS], BF16)
            nc.gpsimd.dma_start(out=embT_a[:], in_=embT[:, 0:HS])
            nc.sync.dma_start(out=embT_b[:], in_=embT[:, HS:])
            # embc triggers on the scalar ring, ahead of any ACT compute
            HC = KCH * D // 2
            embc_a = constp.tile([P, HC], BF16)
            embc_b = constp.tile([P, HC], BF16)
            nc.gpsimd.dma_start(out=embc_a[:], in_=embc[:, 0:HC])
            nc.scalar.dma_start(out=embc_b[:], in_=embc[:, HC:])

            def embT_cols(lo, width):
                if lo < HS:
                    return embT_a[:, lo : lo + width]
                return embT_b[:, lo - HS : lo - HS + width]

            negc_sb = constp.tile([P, 1], F32)
            nc.vector.memset(negc_sb[:], -C_SHIFT)
            # PE p-state warmup: dummy matmuls on scratch data keep the PE
            # continuously busy through the DMA wait so the clock is at full
            # speed when the real stream starts
            warm = constp.tile([P, 512], BF16)
            nc.vector.memset(warm[:], 0.0)
            for w in range(40):
                pw = php.tile([P, 512], F32, tag="ph", name=f"warm{w}")
                nc.tensor.matmul(
                    out=pw[:], lhsT=warm[:, 0:P], rhs=warm[:],
                    start=True, stop=True,
                )

            th_sb = constp.tile([P, N_SPOTS], BF16)  # tanh(W h + b).T  [j, s]
            e_sb = constp.tile([P, KCH], F32)  # e in spot-partition layout
            # per-group X tiles: [Xhi | Xlo] merged per chunk, fp8
            xmg = [
                constp.tile([P, GS * 2 * NX], F8, name=f"xmg{g}") for g in range(NG)
            ]

            # ---- mask slab DMAs, all issued up front (T resident bufs) ----
            mts = []
            for t in range(T):
                mt = maskp.tile([P, KCH * P], F8, name=f"mt{t}", tag="mt")
                nc.gpsimd.dma_start(out=mt[:], in_=maskf[t])
                mts.append(mt)

            # ---- prologue: th = tanh(W@emb.T + b), logits, e ----
            NCH = N_SPOTS // 512  # 8 th chunks of 512 spots
            pe = pep.tile([P, KCH], F32)
            for c in range(NCH):
                ph = php.tile([P, 512], F32, tag="ph")
                nc.tensor.matmul(
                    out=ph[:], lhsT=wt_sb[:], rhs=embT_cols(c * 512, 512),
                    start=True, stop=True,
                )
                nc.scalar.activation(
                    out=th_sb[:, ts(c, 512)], in_=ph[:],
                    func=mybir.ActivationFunctionType.Tanh, bias=b_sb[:, 0:1],
                )
                # logits in spot-partition layout: [128 s, 1] = th_k.T @ v
                for k in range(4 * c, 4 * c + 4):
                    nc.tensor.matmul(
                        out=pe[:, k : k + 1], lhsT=th_sb[:, ts(k, P)], rhs=v_sb[:],
                        start=True, stop=True,
                    )
                nc.scalar.activation(
                    out=e_sb[:, 4 * c : 4 * c + 4], in_=pe[:, 4 * c : 4 * c + 4],
                    func=mybir.ActivationFunctionType.Exp, bias=negc_sb[:, 0:1],
                )

            # ---- X = [e*emb | e] as fp8 hi + 64x lo, 8 groups of 4 chunks ----
            # Pool: mul + lo-cast; ACT: hi-cast; DVE: sub
            xf = xfp.tile([P, KCH * NX], F32)
            xd = xfp.tile([P, KCH * NX], F32)
            xf3 = xf[:].rearrange("p (k n) -> p k n", n=NX)
            xd3 = xd[:].rearrange("p (k n) -> p k n", n=NX)
            emb3a = embc_a[:].rearrange("p (k d) -> p k d", d=D)
            emb3b = embc_b[:].rearrange("p (k d) -> p k d", d=D)
            e3 = e_sb[:].rearrange("p k -> p k ()")
            for g in range(NG):
                ks = slice(g * GS, (g + 1) * GS)
                if g < NG // 2:
                    embsrc = emb3a[:, ks, :]
                else:
                    embsrc = emb3b[:, slice(g * GS - KCH // 2, (g + 1) * GS - KCH // 2), :]
                ebc = e3[:, ks, :].to_broadcast([P, GS, D])
                xg3 = xmg[g][:].rearrange("p (c n) -> p c n", n=2 * NX)
                hi3 = xg3[:, :, 0:NX]
                lo3 = xg3[:, :, NX : 2 * NX]
                nc.vector.tensor_mul(out=xf3[:, ks, 0:D], in0=embsrc, in1=ebc)
                nc.vector.tensor_copy(out=xf3[:, ks, D : D + 1], in_=e3[:, ks, :])
                nc.scalar.activation(
                    out=hi3, in_=xf3[:, ks, :], func=mybir.ActivationFunctionType.Copy
                )
                nc.vector.tensor_sub(out=xd3[:, ks, :], in0=xf3[:, ks, :], in1=hi3)
                nc.scalar.activation(
                    out=lo3, in_=xd3[:, ks, :],
                    func=mybir.ActivationFunctionType.Copy, scale=LO_SCALE,
                )

            # ---- main loop: TG gene tiles per PSUM generation ----
            for tg in range(T // TG):
                tls = list(range(tg * TG, (tg + 1) * TG))
                pts = [
                    ptp.tile([P, 2 * NX], F32, name=f"pt{t}", tag="pt") for t in tls
                ]
                for kk in range(KK):
                    g, kkl = kk // 2, kk % 2
                    xg4 = xmg[g][:].rearrange(
                        "p (kkl i n) -> p kkl i n", i=2, n=2 * NX
                    )
                    rhs = xg4[:, kkl]
                    for i, t in enumerate(tls):
                        mt4 = mts[t][:].rearrange("p (kk j i) -> p kk j i", i=2, j=P)
                        nc.tensor.matmul(
                            out=pts[i][:], lhsT=mt4[:, kk], rhs=rhs,
                            start=(kk == 0), stop=(kk == KK - 1),
                            perf_mode=mybir.MatmulPerfMode.DoubleRowSwInterleave,
                        )
                for i, t in enumerate(tls):
                    pt = pts[i]
                    # s = hi + lo/64 (ACT rescales lo out of PSUM, DVE adds)
                    s1 = outp.tile([P, NX], F32, tag="s1")
                    nc.scalar.activation(
                        out=s1[:], in_=pt[:, NX : 2 * NX],
                        func=mybir.ActivationFunctionType.Copy, scale=1.0 / LO_SCALE,
                    )
                    s2 = outp.tile([P, NX], F32, tag="s2")
                    nc.vector.tensor_add(out=s2[:], in0=s1[:], in1=pt[:, 0:NX])
                    rmax = outp.tile([P, 1], F32, tag="rmax")
                    nc.vector.tensor_scalar_max(
                        out=rmax[:], in0=s2[:, D : D + 1], scalar1=1e-37
                    )
                    rinv = outp.tile([P, 1], F32, tag="rinv")
                    nc.vector.reciprocal(out=rinv[:], in_=rmax[:])
                    o = outp.tile([P, D], F32, tag="o")
                    nc.vector.tensor_scalar_mul(
                        out=o[:], in0=s2[:, 0:D], scalar1=rinv[:, 0:1]
                    )
                    nc.sync.dma_start(out=out[t], in_=o[:])

    nc.compile()
    return nc


def prep_inputs(spot_emb, W, b, v, gene_ids, spot_ids, T):
    """Host marshaling: shared bf16/f32 operands + per-core fp8 mask slabs."""
    emb = np.ascontiguousarray(np.asarray(spot_emb, dtype=np.float32))
    W = np.asarray(W, dtype=np.float32)
    b = np.asarray(b, dtype=np.float32)
    v = np.asarray(v, dtype=np.float32)
    gene_ids = np.asarray(gene_ids).astype(np.int64)
    spot_ids = np.asarray(spot_ids).astype(np.int64)

    bf = ml_dtypes.bfloat16
    shared = {
        "embc": np.ascontiguousarray(
            emb.reshape(KCH, P, D).transpose(1, 0, 2).reshape(P, KCH * D).astype(bf)
        ),
        "embT": np.ascontiguousarray(emb.T.astype(bf)),
        "wt": np.ascontiguousarray(W.T.astype(bf)),
        "bb": np.ascontiguousarray(b.reshape(D, 1)),
        "vv": np.ascontiguousarray(v.reshape(D, 1).astype(bf)),
    }

    # Dense 0/1 occupancy mask (set semantics: duplicate edges collapse),
    # built directly in the per-core padded layout: core c's genes live at
    # rows [c*T*P, c*T*P + G_PER); rows above G_PER stay zero padding.
    g_pad = T * P
    M = np.zeros((N_CORES * g_pad, N_SPOTS), dtype=bool)
    pad_rows = (gene_ids // G_PER) * g_pad + (gene_ids % G_PER)
    M[pad_rows, spot_ids] = True
    # [c, t*128+g, kk*256 + i*128 + p] -> [c, t, p, kk, jrev, i] where the
    # lhsT free layout per kk is pair-interleaved with gene columns reversed
    # (DoubleRowSwInterleave): flat index 2j+i holds ktile i, gene 127-j
    Mbt = M.reshape(N_CORES, T, P, KK, 2, P)[:, :, ::-1].transpose(0, 1, 5, 3, 2, 4)
    # value 1.0 in fp8e4m3 is byte 0x38
    Mf8 = (np.ascontiguousarray(Mbt).astype(np.uint8) * 0x38).view(
        ml_dtypes.float8_e4m3
    ).reshape(N_CORES, T, P, KCH * P)
    return [{"maskf": Mf8[c], **shared} for c in range(N_CORES)]


_NC_CACHE = {}


def run(spot_emb, W, b, v, gene_ids, spot_ids, trace=False, **hw_kwargs):
    T = (G_PER + P - 1) // P  # 20
    if T not in _NC_CACHE:
        nc = build_nc(T)
        nc.m = get_hw_module(nc.m)
        _NC_CACHE[T] = nc
    nc = _NC_CACHE[T]
    in_maps = prep_inputs(spot_emb, W, b, v, gene_ids, spot_ids, T)
    res = run_bass_kernel_spmd(
        nc, in_maps, core_ids=list(range(N_CORES)), trace=trace, **hw_kwargs
    )
    outs = [
        np.asarray(res.results[c]["out"], dtype=np.float32).reshape(T * P, D)[:G_PER]
        for c in range(N_CORES)
    ]
    full = np.concatenate(outs, axis=0)
    return full, res


def kernel(spot_emb, W, b, v, gene_ids, spot_ids, n_genes):
    n_genes = int(n_genes)
    assert n_genes == N_GENES, f"kernel hardcodes n_genes={N_GENES}, got {n_genes}"
    full, _ = run(spot_emb, W, b, v, gene_ids, spot_ids, trace=False)
    return full


# revision 25
# speedup vs baseline: 1.3925x; 1.0111x over previous
"""Trainium2 Bass kernel for nn_AttentionPool (segment softmax-pool over gene/spot edges).

Math: out[g] = (sum_{s in S_g} e_s * emb[s]) / (sum_{s in S_g} e_s),
      e_s = exp(logit_s - c),  logit = tanh(emb @ W.T + b) @ v
where S_g is the *set* of distinct spots expressing gene g (duplicate edges
count once), and empty genes produce 0. Any shift c cancels in the ratio;
c = 5.0 (> max logit ~3.96 for this problem's xavier init) keeps every e_s
in fp8e4m3 range with wide margin on both ends.

Sharding: 2500 genes per core x 8 cores (padded to 2560 = 20 tiles of 128).
Host marshals the edge list into each core's dense fp8 {0,1} mask slab in the
DoubleRowSwInterleave weight layout (pair-interleaved ktiles, gene columns
reversed) so each [128,2,128] chunk is a K=256 matmul lhsT at 0.5 cycles/row.
X = [e*emb | e] is carried as fp8 hi plus 64x-scaled lo residual, merged as
258 rhs columns per chunk so one LDWEIGHTS serves both; the two PSUM column
blocks are recombined as hi + lo/64, giving bf16-level accuracy at 2x rate.
Main loop runs 4 gene tiles per PSUM generation so the tensor engine can
interleave across tiles while X groups are still being produced.
"""

import sys

sys.path.insert(0, "/opt/trn_rl_repo")

import numpy as np
import ml_dtypes

import concourse.mybir as mybir
import concourse.tile as tile
from concourse import bacc
from concourse.bass import ts
from concourse.bass_utils import run_bass_kernel_spmd
from concourse.bass_interp import get_hw_module

F32 = mybir.dt.float32
BF16 = mybir.dt.bfloat16
F8 = mybir.dt.float8e4

N_SPOTS = 4096
N_GENES = 20000
D = 128
N_CORES = 8
G_PER = N_GENES // N_CORES  # 2500
P = 128
KCH = N_SPOTS // P  # 32 spot chunks of 128
KK = KCH // 2  # 16 double-chunks of 256 for DoubleRow
NX = D + 1  # X columns: [e*emb | e]
LO_SCALE = 64.0  # keeps fp8 lo residuals out of the subnormal range
C_SHIFT = 5.0  # logit shift; exact value cancels in the num/den ratio
NG = 8  # X built in 8 groups of 4 chunks
GS = KCH // NG  # 4
TG = 4  # gene tiles per PSUM generation


def build_nc(T):
    """Build the single-core Bass program (SPMD across 8 cores).

    T = number of 128-gene tiles per core (20 for the real problem).
    """
    nc = bacc.Bacc("TRN2", target_bir_lowering=False, debug=False, num_devices=N_CORES)

    maskf = nc.dram_tensor("maskf", [T, P, KCH * P], F8, kind="ExternalInput")
    embT = nc.dram_tensor("embT", [P, N_SPOTS], BF16, kind="ExternalInput")
    embc = nc.dram_tensor("embc", [P, KCH * D], BF16, kind="ExternalInput")
    wt = nc.dram_tensor("wt", [D, D], BF16, kind="ExternalInput")
    bb = nc.dram_tensor("bb", [D, 1], F32, kind="ExternalInput")
    vv = nc.dram_tensor("vv", [D, 1], BF16, kind="ExternalInput")
    out = nc.dram_tensor("out", [T, P, D], F32, kind="ExternalOutput")

    with tile.TileContext(nc) as tc:
        with (
            tc.tile_pool(name="const", bufs=1) as constp,
            tc.tile_pool(name="xfp", bufs=1) as xfp,
            tc.tile_pool(name="maskp", bufs=T) as maskp,
            tc.tile_pool(name="outp", bufs=3) as outp,
            tc.tile_pool(name="php", bufs=2, space="PSUM") as php,
            tc.tile_pool(name="pep", bufs=1, space="PSUM") as pep,
            tc.tile_pool(name="ptp", bufs=TG, space="PSUM") as ptp,
        ):
            # ---- constants into SBUF (sync ring; scalar/ACT stays clean) ----
            wt_sb = constp.tile([P, D], BF16)
            nc.sync.dma_start(out=wt_sb[:], in_=wt[:])
            b_sb = constp.tile([P, 1], F32)
            nc.sync.dma_start(out=b_sb[:], in_=bb[:])
            v_sb = constp.tile([P, 1], BF16)
            nc.sync.dma_start(out=v_sb[:], in_=vv[:])
            HS = N_SPOTS // 2
            embT_a = constp.tile([P, HS], BF16)
            embT_b = constp.tile([P, HS], BF16)
            nc.gpsimd.dma_start(out=embT_a[:], in_=embT[:, 0:HS])
            nc.sync.dma_start(out=embT_b[:], in_=embT[:, HS:])
            # embc triggers on the scalar ring, ahead of any ACT compute
            HC = KCH * D // 2
            embc_a = constp.tile([P, HC], BF16)
            embc_b = constp.tile([P, HC], BF16)
            nc.gpsimd.dma_start(out=embc_a[:], in_=embc[:, 0:HC])
            nc.scalar.dma_start(out=embc_b[:], in_=embc[:, HC:])

            def embT_cols(lo, width):
                if lo < HS:
                    return embT_a[:, lo : lo + width]
                return embT_b[:, lo - HS : lo - HS + width]

            negc_sb = constp.tile([P, 1], F32)
            nc.vector.memset(negc_sb[:], -C_SHIFT)
            # PE p-state warmup: dummy matmuls on scratch data keep the PE
            # continuously busy through the DMA wait so the clock is at full
            # speed when the real stream starts
            warm = constp.tile([P, 512], BF16)
            nc.vector.memset(warm[:], 0.0)
            for w in range(80):
                pw = php.tile([P, 512], F32, tag="ph", name=f"warm{w}")
                nc.tensor.matmul(
                    out=pw[:, 0:64], lhsT=warm[:, 0:P], rhs=warm[:, 0:64],
                    start=True, stop=True,
                )

            th_sb = constp.tile([P, N_SPOTS], BF16)  # tanh(W h + b).T  [j, s]
            e_sb = constp.tile([P, KCH], F32)  # e in spot-partition layout
            # per-group X tiles: [Xhi | Xlo] merged per chunk, fp8
            xmg = [
                constp.tile([P, GS * 2 * NX], F8, name=f"xmg{g}") for g in range(NG)
            ]

            # ---- mask slab DMAs, all issued up front (T resident bufs) ----
            mts = []
            for t in range(T):
                mt = maskp.tile([P, KCH * P], F8, name=f"mt{t}", tag="mt")
                nc.gpsimd.dma_start(out=mt[:], in_=maskf[t])
                mts.append(mt)

            # ---- prologue: th = tanh(W@emb.T + b), logits, e ----
            NCH = N_SPOTS // 512  # 8 th chunks of 512 spots
            pe = pep.tile([P, KCH], F32)
            for c in range(NCH):
                ph = php.tile([P, 512], F32, tag="ph")
                nc.tensor.matmul(
                    out=ph[:], lhsT=wt_sb[:], rhs=embT_cols(c * 512, 512),
                    start=True, stop=True,
                )
                nc.scalar.activation(
                    out=th_sb[:, ts(c, 512)], in_=ph[:],
                    func=mybir.ActivationFunctionType.Tanh, bias=b_sb[:, 0:1],
                )
                # logits in spot-partition layout: [128 s, 1] = th_k.T @ v
                for k in range(4 * c, 4 * c + 4):
                    nc.tensor.matmul(
                        out=pe[:, k : k + 1], lhsT=th_sb[:, ts(k, P)], rhs=v_sb[:],
                        start=True, stop=True,
                    )
                nc.scalar.activation(
                    out=e_sb[:, 4 * c : 4 * c + 4], in_=pe[:, 4 * c : 4 * c + 4],
                    func=mybir.ActivationFunctionType.Exp, bias=negc_sb[:, 0:1],
                )

            # ---- X = [e*emb | e] as fp8 hi + 64x lo, 8 groups of 4 chunks ----
            # Pool: mul + lo-cast; ACT: hi-cast; DVE: sub
            xf = xfp.tile([P, KCH * NX], F32)
            xd = xfp.tile([P, KCH * NX], F32)
            xf3 = xf[:].rearrange("p (k n) -> p k n", n=NX)
            xd3 = xd[:].rearrange("p (k n) -> p k n", n=NX)
            emb3a = embc_a[:].rearrange("p (k d) -> p k d", d=D)
            emb3b = embc_b[:].rearrange("p (k d) -> p k d", d=D)
            e3 = e_sb[:].rearrange("p k -> p k ()")
            for g in range(NG):
                ks = slice(g * GS, (g + 1) * GS)
                if g < NG // 2:
                    embsrc = emb3a[:, ks, :]
                else:
                    embsrc = emb3b[:, slice(g * GS - KCH // 2, (g + 1) * GS - KCH // 2), :]
                ebc = e3[:, ks, :].to_broadcast([P, GS, D])
                xg3 = xmg[g][:].rearrange("p (c n) -> p c n", n=2 * NX)
                hi3 = xg3[:, :, 0:NX]
                lo3 = xg3[:, :, NX : 2 * NX]
                nc.vector.tensor_mul(out=xf3[:, ks, 0:D], in0=embsrc, in1=ebc)
                nc.vector.tensor_copy(out=xf3[:, ks, D : D + 1], in_=e3[:, ks, :])
                nc.scalar.activation(
                    out=hi3, in_=xf3[:, ks, :], func=mybir.ActivationFunctionType.Copy
                )
                nc.vector.tensor_sub(out=xd3[:, ks, :], in0=xf3[:, ks, :], in1=hi3)
                nc.scalar.activation(
                    out=lo3, in_=xd3[:, ks, :],
                    func=mybir.ActivationFunctionType.Copy, scale=LO_SCALE,
                )

            # ---- main loop: TG gene tiles per PSUM generation ----
            for tg in range(T // TG):
                tls = list(range(tg * TG, (tg + 1) * TG))
                pts = [
                    ptp.tile([P, 2 * NX], F32, name=f"pt{t}", tag="pt") for t in tls
                ]
                for kk in range(KK):
                    g, kkl = kk // 2, kk % 2
                    xg4 = xmg[g][:].rearrange(
                        "p (kkl i n) -> p kkl i n", i=2, n=2 * NX
                    )
                    rhs = xg4[:, kkl]
                    for i, t in enumerate(tls):
                        mt4 = mts[t][:].rearrange("p (kk j i) -> p kk j i", i=2, j=P)
                        nc.tensor.matmul(
                            out=pts[i][:], lhsT=mt4[:, kk], rhs=rhs,
                            start=(kk == 0), stop=(kk == KK - 1),
                            perf_mode=mybir.MatmulPerfMode.DoubleRowSwInterleave,
                        )
                for i, t in enumerate(tls):
                    pt = pts[i]
                    # s = hi + lo/64 (ACT rescales lo out of PSUM, DVE adds)
                    s1 = outp.tile([P, NX], F32, tag="s1")
                    nc.scalar.activation(
                        out=s1[:], in_=pt[:, NX : 2 * NX],
                        func=mybir.ActivationFunctionType.Copy, scale=1.0 / LO_SCALE,
                    )
                    s2 = outp.tile([P, NX], F32, tag="s2")
                    nc.vector.tensor_add(out=s2[:], in0=s1[:], in1=pt[:, 0:NX])
                    rmax = outp.tile([P, 1], F32, tag="rmax")
                    nc.vector.tensor_scalar_max(
                        out=rmax[:], in0=s2[:, D : D + 1], scalar1=1e-37
                    )
                    rinv = outp.tile([P, 1], F32, tag="rinv")
                    nc.vector.reciprocal(out=rinv[:], in_=rmax[:])
                    o = outp.tile([P, D], F32, tag="o")
                    nc.vector.tensor_scalar_mul(
                        out=o[:], in0=s2[:, 0:D], scalar1=rinv[:, 0:1]
                    )
                    nc.sync.dma_start(out=out[t], in_=o[:])

    nc.compile()
    return nc


def prep_inputs(spot_emb, W, b, v, gene_ids, spot_ids, T):
    """Host marshaling: shared bf16/f32 operands + per-core fp8 mask slabs."""
    emb = np.ascontiguousarray(np.asarray(spot_emb, dtype=np.float32))
    W = np.asarray(W, dtype=np.float32)
    b = np.asarray(b, dtype=np.float32)
    v = np.asarray(v, dtype=np.float32)
    gene_ids = np.asarray(gene_ids).astype(np.int64)
    spot_ids = np.asarray(spot_ids).astype(np.int64)

    bf = ml_dtypes.bfloat16
    shared = {
        "embc": np.ascontiguousarray(
            emb.reshape(KCH, P, D).transpose(1, 0, 2).reshape(P, KCH * D).astype(bf)
        ),
        "embT": np.ascontiguousarray(emb.T.astype(bf)),
        "wt": np.ascontiguousarray(W.T.astype(bf)),
        "bb": np.ascontiguousarray(b.reshape(D, 1)),
        "vv": np.ascontiguousarray(v.reshape(D, 1).astype(bf)),
    }

    # Dense 0/1 occupancy mask (set semantics: duplicate edges collapse),
    # built directly in the per-core padded layout: core c's genes live at
    # rows [c*T*P, c*T*P + G_PER); rows above G_PER stay zero padding.
    g_pad = T * P
    M = np.zeros((N_CORES * g_pad, N_SPOTS), dtype=bool)
    pad_rows = (gene_ids // G_PER) * g_pad + (gene_ids % G_PER)
    M[pad_rows, spot_ids] = True
    # [c, t*128+g, kk*256 + i*128 + p] -> [c, t, p, kk, jrev, i] where the
    # lhsT free layout per kk is pair-interleaved with gene columns reversed
    # (DoubleRowSwInterleave): flat index 2j+i holds ktile i, gene 127-j
    Mbt = M.reshape(N_CORES, T, P, KK, 2, P)[:, :, ::-1].transpose(0, 1, 5, 3, 2, 4)
    # value 1.0 in fp8e4m3 is byte 0x38
    Mf8 = (np.ascontiguousarray(Mbt).astype(np.uint8) * 0x38).view(
        ml_dtypes.float8_e4m3
    ).reshape(N_CORES, T, P, KCH * P)
    return [{"maskf": Mf8[c], **shared} for c in range(N_CORES)]


_NC_CACHE = {}


def run(spot_emb, W, b, v, gene_ids, spot_ids, trace=False, **hw_kwargs):
    T = (G_PER + P - 1) // P  # 20
    if T not in _NC_CACHE:
        nc = build_nc(T)
        nc.m = get_hw_module(nc.m)
        _NC_CACHE[T] = nc
    nc = _NC_CACHE[T]
    in_maps = prep_inputs(spot_emb, W, b, v, gene_ids, spot_ids, T)
    res = run_bass_kernel_spmd(
        nc, in_maps, core_ids=list(range(N_CORES)), trace=trace, **hw_kwargs
    )
    outs = [
        np.asarray(res.results[c]["out"], dtype=np.float32).reshape(T * P, D)[:G_PER]
        for c in range(N_CORES)
    ]
    full = np.concatenate(outs, axis=0)
    return full, res


def kernel(spot_emb, W, b, v, gene_ids, spot_ids, n_genes):
    n_genes = int(n_genes)
    assert n_genes == N_GENES, f"kernel hardcodes n_genes={N_GENES}, got {n_genes}"
    full, _ = run(spot_emb, W, b, v, gene_ids, spot_ids, trace=False)
    return full
